# revision 10
# baseline (speedup 1.0000x reference)
"""Trainium2 Bass kernel for nn_Graph_Generator (retrieval_knn).

Computes per-view pairwise-Euclidean kNN (k=16) graphs for V=3 views of
[N=8192, D=128] features, sharded across 8 NeuronCores by query rows
(1024 rows/core, each core scans all N columns).

Device kernel per core, per view, per 128-query block, per 512-col chunk:
  - PE: fp16 K=2 matmul accumulating -sq_j (hi+lo split, ~6e-5 abs exact;
        runs FIRST so its separate LDWEIGHTS carries the cross-engine
        waits -- fp32 matmuls embed the weight load and can take only one)
        then fp32 matmul psum += (2*Xq^T).T @ X^T chunk
  - ACT: evacuate psum -> SBUF with per-partition bias -sq_q,
         giving m = 2g - sq_q - sq_j = -d^2  (top-k on m == top-k on -d)
  - DVE: chunked max8 candidates (top-8 per 256 cols), top-16 of
         candidates (max8 + match_replace + max8), two max_index passes
         over the full [128, 8192] row for exact indices
  - ACT: d16 = sqrt(relu(-m16)); DMA out dist + indices.

X^T / 2*Xq^T are pre-transposed on the host (free) and DMA'd straight in.
Host side: shards inputs, gathers per-core outputs, builds the
deterministic iota-style edge tensors, passes through inputs.
"""

import numpy as np

V, N, D, L, K = 3, 8192, 128, 64, 16
M_CORES = 8
NQ = N // M_CORES          # 1024 query rows per core
P = 128                    # SBUF partitions
MMCHUNK = 512              # matmul free-dim chunk (one PSUM bank of fp32)
CCHUNK = 512               # stage-1 candidate chunk (top-8 per chunk)
NEG_INF = -3.0e38

_BASS_CACHE = {}


def _build_bass(n_cols, nq):
    """Build the Bass program. n_cols = N (columns scanned), nq = rows/core."""
    import concourse.bacc as bacc
    import concourse.mybir as mybir
    from concourse.tile import TileContext

    f32 = mybir.dt.float32
    f16 = mybir.dt.float16
    i32 = mybir.dt.int32
    u32 = mybir.dt.uint32
    AF = mybir.ActivationFunctionType

    n_blocks = nq // P
    n_mm = n_cols // MMCHUNK
    n_cc = n_cols // CCHUNK

    nc = bacc.Bacc()
    # X^T columns [0:n_cols) and 2*Xq^T columns [n_cols:n_cols+nq), one DMA
    xtall_in = nc.dram_tensor(
        "xtall_in", [V, D, n_cols + nq], f32, kind="ExternalInput"
    )
    # [V, 2, n_cols + 128]: cols [0:n) = -sq_j hi/lo fp16; cols [n:) = 1.0
    negsq_hl = nc.dram_tensor(
        "negsq_hl", [V, 2, n_cols + P], f16, kind="ExternalInput"
    )
    negsq_q = nc.dram_tensor("negsq_q", [V, nq], f32, kind="ExternalInput")
    dist_out = nc.dram_tensor("dist_out", [V, nq, K], f32, kind="ExternalOutput")
    idx_out = nc.dram_tensor("idx_out", [V, nq, K], i32, kind="ExternalOutput")

    with TileContext(nc) as tc:
        with (
            tc.tile_pool(name="xt", bufs=2) as xt_pool,
            tc.tile_pool(name="nsq", bufs=2) as nsq_pool,
            tc.tile_pool(name="mrow", bufs=2) as m_pool,
            tc.tile_pool(name="cand", bufs=2) as cand_pool,
            tc.tile_pool(name="small", bufs=2) as small_pool,
            tc.tile_pool(name="mm_psum", bufs=4, space="PSUM") as mm_psum,
        ):
            for v in range(V):
                xtall = xt_pool.tile([P, n_cols + nq], f32)  # [X^T | 2*Xq^T]
                nsq = nsq_pool.tile([2, n_cols + P], f16)    # [-sq_j hi; lo | ones]
                nsqq = nsq_pool.tile([P, n_blocks], f32, tag="nsqq")

                nc.sync.dma_start(out=xtall, in_=xtall_in[v])
                nc.sync.dma_start(out=nsq, in_=negsq_hl[v])
                xt = xtall[:, 0:n_cols]
                xq2 = xtall[:, n_cols:n_cols + nq]
                nc.sync.dma_start(
                    out=nsqq, in_=negsq_q[v].rearrange("(b p) -> p b", p=P)
                )
                ones2 = nsq[:, n_cols:n_cols + P]            # [2, 128] of 1.0h
                # ACT-engine toucher: absorbs the nsqq DMA wait so the hot
                # psum-evac activations carry only the PE wait (walrus allows
                # a single sync wait per instruction here).
                nsqq2 = nsq_pool.tile([P, n_blocks], f32, tag="nsqq2")
                nc.scalar.copy(out=nsqq2, in_=nsqq)

                for b in range(n_blocks):
                    m_row = m_pool.tile([P, n_cols], f32)
                    lhsT = xq2[:, b * P:(b + 1) * P]
                    bias = nsqq2[:, b:b + 1]
                    for c in range(n_mm):
                        sl = slice(c * MMCHUNK, (c + 1) * MMCHUNK)
                        ps = mm_psum.tile([P, MMCHUNK], f32)
                        # fp16 fold first: its LDWEIGHTS absorbs the waits
                        nc.tensor.matmul(
                            ps, ones2, nsq[:, sl], start=True, stop=False,
                            skip_group_check=True,
                        )
                        nc.tensor.matmul(
                            ps, lhsT, xt[:, sl], start=False, stop=True,
                            skip_group_check=True,
                        )
                        # m = psum + (-sq_q)  (per-partition bias)
                        nc.scalar.activation(
                            out=m_row[:, sl], in_=ps, func=AF.Identity, bias=bias
                        )

                    # stage 1: top-8 of each 256-col chunk -> candidates
                    cand = cand_pool.tile([P, 8 * n_cc], f32, tag="cand")
                    for c in range(n_cc):
                        nc.vector.max(
                            out=cand[:, 8 * c:8 * c + 8],
                            in_=m_row[:, c * CCHUNK:(c + 1) * CCHUNK],
                        )
                    # stage 2: exact top-16 values from candidates
                    v16 = small_pool.tile([P, K], f32, tag="v16")
                    vhi, vlo = v16[:, 0:8], v16[:, 8:16]
                    cand2 = cand_pool.tile([P, 8 * n_cc], f32, tag="cand2")
                    nc.vector.max(out=vhi, in_=cand)
                    nc.vector.match_replace(
                        out=cand2, in_to_replace=vhi, in_values=cand,
                        imm_value=NEG_INF,
                    )
                    nc.vector.max(out=vlo, in_=cand2)
                    # stage 3: indices via two full-row find_index8 passes
                    i16u = small_pool.tile([P, K], u32, tag="i16u")
                    nc.vector.max_index(i16u[:, 0:8], vhi, m_row)
                    nc.vector.max_index(i16u[:, 8:16], vlo, m_row)

                    # d = sqrt(relu(-m)) on ACT, keeping DVE free
                    t16 = small_pool.tile([P, K], f32, tag="t16")
                    nc.scalar.activation(out=t16, in_=v16, func=AF.Relu, scale=-1.0)
                    d16 = small_pool.tile([P, K], f32, tag="d16")
                    nc.scalar.activation(out=d16, in_=t16, func=AF.Sqrt)

                    nc.sync.dma_start(
                        out=dist_out[v, b * P:(b + 1) * P, :], in_=d16
                    )
                    # uint32 -> int32 is a bitwise no-op for idx < 2^31
                    nc.sync.dma_start(
                        out=idx_out[v, b * P:(b + 1) * P, :],
                        in_=i16u.bitcast(i32),
                    )
    nc.finalize()
    return nc


def _get_bass(n_cols, nq):
    key = (n_cols, nq)
    if key not in _BASS_CACHE:
        _BASS_CACHE[key] = _build_bass(n_cols, nq)
    return _BASS_CACHE[key]


def run_device(train_feats, n_cores=M_CORES, **spmd_kwargs):
    """Run the sharded device kernel. Returns (dist [V,N,K] f32, idx [V,N,K] i32)."""
    from concourse.bass_utils import run_bass_kernel_spmd

    x = np.ascontiguousarray(train_feats, dtype=np.float32)
    v, n, d = x.shape
    nq = n // n_cores
    negsq = -np.sum(x * x, axis=-1)                    # [V, N] f32
    hi = negsq.astype(np.float16)
    lo = (negsq - hi.astype(np.float32)).astype(np.float16)
    negsq_hl = np.ones((v, 2, n + P), dtype=np.float16)
    negsq_hl[:, 0, :n] = hi
    negsq_hl[:, 1, :n] = lo

    xt_host = np.ascontiguousarray(x.transpose(0, 2, 1))          # [V, D, N]

    nc = _get_bass(n, nq)
    in_maps = []
    for c in range(n_cores):
        sl = slice(c * nq, (c + 1) * nq)
        in_maps.append({
            "xtall_in": np.ascontiguousarray(
                np.concatenate([xt_host, 2.0 * xt_host[:, :, sl]], axis=2)
            ),
            "negsq_hl": negsq_hl,
            "negsq_q": np.ascontiguousarray(negsq[:, sl]),
        })
    res = run_bass_kernel_spmd(nc, in_maps, core_ids=list(range(n_cores)),
                               **spmd_kwargs)
    dist = np.concatenate([r["dist_out"] for r in res.results], axis=1)
    idx = np.concatenate([r["idx_out"] for r in res.results], axis=1)
    return dist, idx, res


def kernel(train_feats, label_feats, target, k):
    assert int(k) == K
    train_feats = np.asarray(train_feats, dtype=np.float32)
    label_feats = np.asarray(label_feats, dtype=np.float32)
    v, n, d = train_feats.shape
    num_label = label_feats.shape[0]

    topk_dist, topk_index, _ = run_device(train_feats)

    src = np.repeat(np.arange(n, dtype=np.int32), K)
    dst = topk_index.reshape(v, n * K).astype(np.int32)
    edge_idx_ins = np.stack(
        [np.broadcast_to(src, dst.shape), dst], axis=1
    ).astype(np.int32)                                  # [V, 2, N*K]

    ins_idx_C = np.repeat(np.arange(n, dtype=np.int32), v)
    view_idx_C = np.tile(np.arange(v, dtype=np.int32), n)
    view_edge_index_C = np.stack([ins_idx_C, view_idx_C], axis=1)

    gins_idx_C = np.repeat(np.arange(n, dtype=np.int32), num_label)
    label_idx_C = np.tile(np.arange(num_label, dtype=np.int32), n)
    gedge_index_C = np.stack([gins_idx_C, label_idx_C], axis=1)

    return (topk_dist, edge_idx_ins, train_feats, label_feats,
            view_edge_index_C, gedge_index_C)
